# revision 1
# baseline (speedup 1.0000x reference)
"""Trainium2 Bass kernel for nn_RNN_7842610283034.

RNN: 128 warmup steps (teacher-forced) + 128 autoregressive steps.
  cell:  h' = relu(x @ W_ih^T + b_ih + h @ W_hh^T + b_hh)
  fc:    pred = h @ W_fc^T + b_fc

Strategy (data-parallel over batch, 8 cores x 32 batch):
  * Algebraic fusion of the AR feedback path on the host:
        x_{t+1} = pred_t  =>  h' = relu(h @ W_ar^T + b_ar)
        W_ar = W_hh + W_ih @ W_fc,  b_ar = b_ih + b_hh + W_ih @ b_fc
    so the AR phase is a single 2048x2048 recurrence and all fc
    projections batch into one big matmul at the end.
  * All activations kept transposed on device ([hid, batch], hid on
    partitions); weights pre-transposed on host so they load directly
    as PE stationary tiles.
  * fp16 everywhere on device (PE runs fp16 at 1 cycle/row vs 4 for
    fp32; recurrence error stays ~3e-4), fp32 PSUM accumulate, fp32 out.
  * Phase 1: z_t = x_t @ W_ih^T + (b_ih+b_hh) for all 128 warmup steps
    as one batched matmul -> DRAM.
  * Phase 2/3: sequential recurrence, weights resident in SBUF.
  * Phase 4: preds for all 129 stored h_t in one batched matmul.
"""

import sys

sys.path.insert(0, "/opt/trn_rl_repo")

import json

import numpy as np

import concourse.bass as bass
import concourse.mybir as mybir
from concourse.bass_utils import run_bass_kernel_spmd
from concourse.tile import TileContext

# ---------------------------------------------------------------------------
# Workaround for the walrus build in this container: it supports only ONE sem
# wait per instruction ("Too many sync wait commands").  Post-process the BIR
# JSON: hoist extra on_wait entries onto NoOps inserted just before the
# instruction on the same engine (program order keeps this sound).
_orig_to_json_bytes = bass.Bass.to_json_bytes


def _split_multi_waits(m):
    uid = 0
    for fn in m.get("functions", []):
        for blk in fn.get("blocks", []):
            insts = blk.get("instructions")
            if not insts:
                continue
            out = []
            for inst in insts:
                si = inst.get("sync_info")
                if si:
                    ow = si.get("on_wait") or []
                    if len(ow) > 1:
                        eng = inst.get("engine", "SP")
                        for w in ow[:-1]:
                            out.append(
                                {
                                    "engine": eng,
                                    "ins": [],
                                    "name": f"NWS-{uid}",
                                    "opcode": "NoOp",
                                    "outs": [],
                                    "sync_info": {"on_update": [], "on_wait": [w]},
                                }
                            )
                            uid += 1
                        si["on_wait"] = [ow[-1]]
                out.append(inst)
            blk["instructions"] = out


def _patched_to_json_bytes(self, *args, **kwargs):
    data = _orig_to_json_bytes(self, *args, **kwargs)
    m = json.loads(data)
    _split_multi_waits(m)
    return json.dumps(m).encode()


bass.Bass.to_json_bytes = _patched_to_json_bytes
# ---------------------------------------------------------------------------

P = 128          # partitions
B_FULL = 256     # full batch
T_WARM = 128     # warmup steps
T_AR = 128       # autoregressive steps
T_OUT = T_AR + 1
HID = 2048
FEAT = 1024
N_CORES = 8
B = B_FULL // N_CORES  # 32 batch per core

NKH = HID // P   # 16 k-tiles over hid
NMH = HID // P   # 16 m-tiles over hid
NKF = FEAT // P  # 8 k-tiles over feat
NMF = FEAT // P  # 8 m-tiles over feat

FP16 = mybir.dt.float16
F32 = mybir.dt.float32

TC8 = 16         # t-chunk size for batched matmuls (16*32 = 512 free dim)
N_TCH_W = T_WARM // TC8   # 8 warmup t-chunks


def _build_nc():
    nc = bass.Bass()

    x_T = nc.declare_dram_parameter("x_T", [FEAT, T_WARM, B], FP16, isOutput=False)
    w_ih = nc.declare_dram_parameter("w_ih", [FEAT, HID], FP16, isOutput=False)
    w_hh = nc.declare_dram_parameter("w_hh", [HID, HID], FP16, isOutput=False)
    w_ar = nc.declare_dram_parameter("w_ar", [HID, HID], FP16, isOutput=False)
    w_fc = nc.declare_dram_parameter("w_fc", [HID, FEAT], FP16, isOutput=False)
    b_z = nc.declare_dram_parameter("b_z", [P, NMH], F32, isOutput=False)
    b_ar = nc.declare_dram_parameter("b_ar", [P, NMH], F32, isOutput=False)
    b_fc = nc.declare_dram_parameter("b_fc", [P, NMF], F32, isOutput=False)
    out = nc.declare_dram_parameter("out", [FEAT, T_OUT, B], F32, isOutput=True)

    with TileContext(nc) as tc:
        with (
            tc.tile_pool(name="dram", bufs=1, space="DRAM") as dpool,
            tc.tile_pool(name="cpool", bufs=1) as cpool,
        ):
            z_dram = dpool.tile([T_WARM, P, NMH, B], FP16, tag="z")
            h_dram = dpool.tile([T_OUT, P, NMH, B], FP16, tag="h")
            z_dview = z_dram.rearrange("t p m b -> p t m b")
            h_dview = h_dram.rearrange("t p m b -> p t m b")

            bz_sb = cpool.tile([P, NMH], F32, tag="bz")
            nc.sync.dma_start(out=bz_sb[:], in_=b_z[:])
            bar_sb = cpool.tile([P, NMH], F32, tag="bar")
            nc.sync.dma_start(out=bar_sb[:], in_=b_ar[:])
            bfc_sb = cpool.tile([P, NMF], F32, tag="bfc")
            nc.sync.dma_start(out=bfc_sb[:], in_=b_fc[:])

            # ------------- Phase 1: z = x @ W_ih^T + b_z (transposed) -------
            with (
                tc.tile_pool(name="p1w", bufs=1) as p1w,
                tc.tile_pool(name="p1x", bufs=12) as p1x,
                tc.tile_pool(name="p1z", bufs=4) as p1z,
                tc.tile_pool(name="p1ps", bufs=4, space="PSUM") as p1ps,
            ):
                wih_sb = p1w.tile([P, NKF * HID], FP16, tag="wih")
                nc.sync.dma_start(
                    out=wih_sb.rearrange("p (k m) -> p k m", k=NKF),
                    in_=w_ih.rearrange("(k p) m -> p k m", p=P),
                )
                x_view = x_T.rearrange("(k p) t b -> p k t b", p=P)
                for tch in range(N_TCH_W):
                    t0 = tch * TC8
                    x_tiles = []
                    for k in range(NKF):
                        xt = p1x.tile([P, TC8 * B], FP16, tag="x")
                        nc.sync.dma_start(
                            out=xt.rearrange("p (t b) -> p t b", t=TC8),
                            in_=x_view[:, k, t0 : t0 + TC8, :],
                        )
                        x_tiles.append(xt)
                    for m in range(NMH):
                        acc = p1ps.tile([P, TC8 * B], F32, tag="zacc")
                        for k in range(NKF):
                            nc.tensor.matmul(
                                acc[:],
                                wih_sb[:, k * HID + m * P : k * HID + (m + 1) * P],
                                x_tiles[k][:],
                                start=(k == 0),
                                stop=(k == NKF - 1),
                            )
                        zt = p1z.tile([P, TC8 * B], FP16, tag="zt")
                        nc.vector.tensor_scalar_add(zt[:], acc[:], bz_sb[:, m : m + 1])
                        nc.sync.dma_start(
                            out=z_dview[:, t0 : t0 + TC8, m, :],
                            in_=zt.rearrange("p (t b) -> p t b", t=TC8),
                        )

            # ------------- Phase 2+3: recurrence ---------------------------
            with (
                tc.tile_pool(name="p2w", bufs=2) as p2w,
                tc.tile_pool(name="p2h", bufs=3) as p2h,
                tc.tile_pool(name="p2z", bufs=4) as p2z,
                tc.tile_pool(name="p2ps", bufs=8, space="PSUM") as p2ps,
            ):
                whh_sb = p2w.tile([P, NKH * HID], FP16, tag="w")
                nc.sync.dma_start(
                    out=whh_sb.rearrange("p (k m) -> p k m", k=NKH),
                    in_=w_hh.rearrange("(k p) m -> p k m", p=P),
                )
                war_sb = p2w.tile([P, NKH * HID], FP16, tag="w")
                nc.sync.dma_start(
                    out=war_sb.rearrange("p (k m) -> p k m", k=NKH),
                    in_=w_ar.rearrange("(k p) m -> p k m", p=P),
                )

                h_prev = p2h.tile([P, NMH * B], FP16, tag="h")
                nc.vector.memset(h_prev[:], 0.0)

                def step(h_prev, wt, z_t=None, store_t=None):
                    h_cur = p2h.tile([P, NMH * B], FP16, tag="h")
                    for m in range(NMH):
                        acc = p2ps.tile([P, B], F32, tag="acc")
                        for k in range(NKH):
                            nc.tensor.matmul(
                                acc[:],
                                wt[:, k * HID + m * P : k * HID + (m + 1) * P],
                                h_prev[:, k * B : (k + 1) * B],
                                start=(k == 0),
                                stop=(k == NKH - 1),
                            )
                        out_sl = h_cur[:, m * B : (m + 1) * B]
                        if z_t is not None:
                            nc.vector.tensor_tensor(
                                out_sl, acc[:], z_t[:, m * B : (m + 1) * B],
                                op=mybir.AluOpType.add,
                            )
                            nc.vector.tensor_scalar_max(out_sl, out_sl, 0.0)
                        else:
                            nc.vector.tensor_scalar(
                                out_sl, acc[:], bar_sb[:, m : m + 1], 0.0,
                                op0=mybir.AluOpType.add, op1=mybir.AluOpType.max,
                            )
                    if store_t is not None:
                        nc.sync.dma_start(out=h_dram[store_t], in_=h_cur.rearrange("p (m b) -> p m b", m=NMH))
                    return h_cur

                for t in range(T_WARM):
                    z_t = p2z.tile([P, NMH * B], FP16, tag="z")
                    nc.sync.dma_start(out=z_t.rearrange("p (m b) -> p m b", m=NMH), in_=z_dram[t])
                    h_prev = step(h_prev, whh_sb, z_t=z_t)

                nc.sync.dma_start(out=h_dram[0], in_=h_prev.rearrange("p (m b) -> p m b", m=NMH))

                for t in range(T_AR):
                    h_prev = step(h_prev, war_sb, z_t=None, store_t=t + 1)

            # ------------- Phase 4: preds = h_hist @ W_fc^T + b_fc ----------
            with (
                tc.tile_pool(name="p4w", bufs=1) as p4w,
                tc.tile_pool(name="p4h", bufs=20) as p4h,
                tc.tile_pool(name="p4o", bufs=4) as p4o,
                tc.tile_pool(name="p4ps", bufs=4, space="PSUM") as p4ps,
            ):
                wfc_sb = p4w.tile([P, NKH * FEAT], FP16, tag="wfc")
                nc.sync.dma_start(
                    out=wfc_sb.rearrange("p (k m) -> p k m", k=NKH),
                    in_=w_fc.rearrange("(k p) m -> p k m", p=P),
                )
                out_view = out.rearrange("(mp p) t b -> p mp t b", p=P)
                # 129 = 8 chunks of 16 + 1
                chunks = [(i * TC8, TC8) for i in range(8)] + [(8 * TC8, 1)]
                for t0, tn in chunks:
                    h_tiles = []
                    for k in range(NKH):
                        ht = p4h.tile([P, TC8 * B], FP16, tag="hh")
                        nc.sync.dma_start(
                            out=ht.rearrange("p (t b) -> p t b", t=TC8)[:, :tn, :],
                            in_=h_dview[:, t0 : t0 + tn, k, :],
                        )
                        h_tiles.append(ht)
                    for mp in range(NMF):
                        acc = p4ps.tile([P, TC8 * B], F32, tag="oacc")
                        for k in range(NKH):
                            nc.tensor.matmul(
                                acc[:, : tn * B],
                                wfc_sb[:, k * FEAT + mp * P : k * FEAT + (mp + 1) * P],
                                h_tiles[k][:, : tn * B],
                                start=(k == 0),
                                stop=(k == NKH - 1),
                            )
                        ot = p4o.tile([P, TC8 * B], F32, tag="ot")
                        nc.vector.tensor_scalar_add(
                            ot[:, : tn * B], acc[:, : tn * B], bfc_sb[:, mp : mp + 1]
                        )
                        nc.sync.dma_start(
                            out=out_view[:, mp, t0 : t0 + tn, :],
                            in_=ot.rearrange("p (t b) -> p t b", t=TC8)[:, :tn, :],
                        )
    return nc


_cached_nc = None


def _get_nc():
    global _cached_nc
    if _cached_nc is None:
        _cached_nc = _build_nc()
    return _cached_nc


def _prep_inputs(inputs, W_ih, b_ih, W_hh, b_hh, W_fc, b_fc):
    inputs = np.asarray(inputs, dtype=np.float32)
    W_ih = np.asarray(W_ih, dtype=np.float32)
    W_hh = np.asarray(W_hh, dtype=np.float32)
    W_fc = np.asarray(W_fc, dtype=np.float32)
    b_ih = np.asarray(b_ih, dtype=np.float32)
    b_hh = np.asarray(b_hh, dtype=np.float32)
    b_fc = np.asarray(b_fc, dtype=np.float32)

    W_ar = W_hh + W_ih @ W_fc
    b_arr = b_ih + b_hh + W_ih @ b_fc
    b_zz = b_ih + b_hh

    common = {
        "w_ih": np.ascontiguousarray(W_ih.T).astype(np.float16),
        "w_hh": np.ascontiguousarray(W_hh.T).astype(np.float16),
        "w_ar": np.ascontiguousarray(W_ar.T).astype(np.float16),
        "w_fc": np.ascontiguousarray(W_fc.T).astype(np.float16),
        "b_z": np.ascontiguousarray(b_zz.reshape(NMH, P).T),
        "b_ar": np.ascontiguousarray(b_arr.reshape(NMH, P).T),
        "b_fc": np.ascontiguousarray(b_fc.reshape(NMF, P).T),
    }
    in_maps = []
    for c in range(N_CORES):
        xc = inputs[c * B : (c + 1) * B]                      # [32, 128, 1024]
        x_T = np.ascontiguousarray(xc.transpose(2, 1, 0)).astype(np.float16)
        in_maps.append({"x_T": x_T, **common})
    return in_maps


def _assemble(results):
    outs = []
    for c in range(N_CORES):
        o = results[c]["out"]                                # [1024, 129, 32] f32
        outs.append(np.ascontiguousarray(o.transpose(2, 1, 0)))  # [32, 129, 1024]
    return np.concatenate(outs, axis=0)


def run(trace=False, **inputs):
    in_maps = _prep_inputs(**inputs)
    nc = _get_nc()
    res = run_bass_kernel_spmd(nc, in_maps, list(range(N_CORES)), trace=trace)
    return _assemble(res.results), res


def kernel(**inputs):
    out, _ = run(trace=False, **inputs)
    return out


# revision 15
# speedup vs baseline: 1.1479x; 1.1479x over previous
"""Trainium2 Bass kernel for nn_RNN_7842610283034.

RNN: 128 warmup steps (teacher-forced) + 128 autoregressive steps.
  cell:  h' = relu(x @ W_ih^T + b_ih + h @ W_hh^T + b_hh)
  fc:    pred = h @ W_fc^T + b_fc

Strategy (data-parallel over batch, 8 cores x 32 batch):
  * Algebraic fusion of the AR feedback path on the host:
        x_{t+1} = pred_t  =>  h' = relu(h @ W_ar^T + b_ar)
        W_ar = W_hh + W_ih @ W_fc,  b_ar = b_ih + b_hh + W_ih @ b_fc
    so the AR phase is a single 2048x2048 recurrence and all fc
    projections batch into one big matmul at the end.
  * All activations kept transposed on device ([hid, batch], hid on
    partitions); weights pre-transposed on host so they load directly
    as PE stationary tiles.
  * fp16 everywhere on device (PE runs fp16 at 1 cycle/row vs 4 for
    fp32; recurrence error stays ~3e-4), fp32 PSUM accumulate, fp32 out.
  * Phase 1: z_t = x_t @ W_ih^T + (b_ih+b_hh) for all 128 warmup steps
    as one batched matmul -> DRAM.
  * Phase 2/3: sequential recurrence, weights resident in SBUF.
  * Phase 4: preds for all 129 stored h_t in one batched matmul.
"""

import sys

sys.path.insert(0, "/opt/trn_rl_repo")

import json

import numpy as np

import concourse.bass as bass
import concourse.mybir as mybir
from concourse.bass_utils import run_bass_kernel_spmd
from concourse.tile import TileContext

# ---------------------------------------------------------------------------
# Workaround for the walrus build in this container: it supports only ONE sem
# wait per instruction ("Too many sync wait commands").  Post-process the BIR
# JSON: hoist extra on_wait entries onto NoOps inserted just before the
# instruction on the same engine (program order keeps this sound).
_orig_to_json_bytes = bass.Bass.to_json_bytes


def _split_multi_waits(m):
    uid = 0
    for fn in m.get("functions", []):
        for blk in fn.get("blocks", []):
            insts = blk.get("instructions")
            if not insts:
                continue
            out = []
            for inst in insts:
                si = inst.get("sync_info")
                if si:
                    ow = si.get("on_wait") or []
                    if len(ow) > 1:
                        eng = inst.get("engine", "SP")
                        for w in ow[:-1]:
                            out.append(
                                {
                                    "engine": eng,
                                    "ins": [],
                                    "name": f"NWS-{uid}",
                                    "opcode": "NoOp",
                                    "outs": [],
                                    "sync_info": {"on_update": [], "on_wait": [w]},
                                }
                            )
                            uid += 1
                        si["on_wait"] = [ow[-1]]
                out.append(inst)
            blk["instructions"] = out


def _strip_dead_pe_incs(m):
    """Drop PE monotonic-sem increments whose tick value no wait references,
    renumbering the remaining waits.  Tile ticks the PE clock sem on EVERY
    matmul (67K incs here) but only accumulation-group boundaries are waited
    on; each EVT_SEM write costs NX time on the hot path."""
    import bisect

    for fn in m.get("functions", []):
        # program order = block order + list order (single linear tile CFG)
        insts = [i for blk in fn.get("blocks", []) for i in blk.get("instructions", [])]
        # engine name -> sem id for single-inc monotonic engine sems on PE
        for eng, sem_name_prefix in (("PE", "PE_"),):
            sem_ids = set()
            for i in insts:
                if i.get("engine") != eng:
                    continue
                for u in ((i.get("sync_info") or {}).get("on_update") or []):
                    if (
                        str(u.get("ant_name", "")).startswith(sem_name_prefix)
                        and u.get("update_mode") == "sem-inc"
                        and u.get("update_value") == 1
                    ):
                        sem_ids.add(u["id"])
            for sem in sem_ids:
                inc_insts = []
                for i in insts:
                    for u in ((i.get("sync_info") or {}).get("on_update") or []):
                        if u.get("id") == sem and u.get("update_mode") == "sem-inc":
                            inc_insts.append((i, u))
                referenced = set()
                ok = True
                for i in insts:
                    for w in ((i.get("sync_info") or {}).get("on_wait") or []):
                        if w.get("id") == sem:
                            if w.get("wait_mode") != "sem-ge-imm":
                                ok = False
                            referenced.add(w.get("wait_value"))
                if not ok or not inc_insts:
                    continue
                kept_ticks = sorted(
                    {t for t in referenced if 1 <= t <= len(inc_insts)}
                    | {len(inc_insts)}
                )
                kept_set = set(kept_ticks)
                for tick, (i, u) in enumerate(inc_insts, start=1):
                    if tick not in kept_set:
                        i["sync_info"]["on_update"].remove(u)
                for i in insts:
                    for w in ((i.get("sync_info") or {}).get("on_wait") or []):
                        if w.get("id") == sem:
                            v = w.get("wait_value")
                            w["wait_value"] = bisect.bisect_right(kept_ticks, v)


def _patched_to_json_bytes(self, *args, **kwargs):
    data = _orig_to_json_bytes(self, *args, **kwargs)
    m = json.loads(data)
    _strip_dead_pe_incs(m)
    _split_multi_waits(m)
    return json.dumps(m).encode()


bass.Bass.to_json_bytes = _patched_to_json_bytes
# ---------------------------------------------------------------------------

P = 128          # partitions
B_FULL = 256     # full batch
T_WARM = 128     # warmup steps
T_AR = 128       # autoregressive steps
T_OUT = T_AR + 1
HID = 2048
FEAT = 1024
N_CORES = 8
B = B_FULL // N_CORES  # 32 batch per core

NKH = HID // P   # 16 k-tiles over hid
NMH = HID // P   # 16 m-tiles over hid
NKF = FEAT // P  # 8 k-tiles over feat
NMF = FEAT // P  # 8 m-tiles over feat

FP16 = mybir.dt.float16
F32 = mybir.dt.float32

TC8 = 16         # t-chunk size for batched matmuls (16*32 = 512 free dim)
N_TCH_W = T_WARM // TC8   # 8 warmup t-chunks


def _build_nc():
    nc = bass.Bass()

    x_T = nc.declare_dram_parameter("x_T", [FEAT, T_WARM, B], FP16, isOutput=False)
    w_ih = nc.declare_dram_parameter("w_ih", [FEAT, HID], FP16, isOutput=False)
    w_hh = nc.declare_dram_parameter("w_hh", [HID, HID], FP16, isOutput=False)
    w_ar = nc.declare_dram_parameter("w_ar", [HID, HID], FP16, isOutput=False)
    w_fc = nc.declare_dram_parameter("w_fc", [HID, FEAT], FP16, isOutput=False)
    b_z = nc.declare_dram_parameter("b_z", [P, NMH], F32, isOutput=False)
    b_ar = nc.declare_dram_parameter("b_ar", [P, NMH], F32, isOutput=False)
    b_fc = nc.declare_dram_parameter("b_fc", [P, NMF], F32, isOutput=False)
    out = nc.declare_dram_parameter("out", [FEAT, T_OUT, B], F32, isOutput=True)

    with TileContext(nc) as tc:
        with (
            tc.tile_pool(name="dram", bufs=1, space="DRAM") as dpool,
            tc.tile_pool(name="cpool", bufs=1) as cpool,
        ):
            # m-major layouts: phase-1 z stores and phase-4 h loads are
            # contiguous per partition; the per-step strided access (z load,
            # h store) hides inside the recurrence.
            z_dram = dpool.tile([NMH, P, T_WARM, B], FP16, tag="z")
            h_dram = dpool.tile([NMH, P, T_OUT, B], FP16, tag="h")
            z_step_view = z_dram.rearrange("m p t b -> p t m b")
            h_step_view = h_dram.rearrange("m p t b -> p t m b")

            bz_sb = cpool.tile([P, NMH], F32, tag="bz")
            nc.sync.dma_start(out=bz_sb[:], in_=b_z[:])
            bar_sb = cpool.tile([P, NMH], F32, tag="bar")
            nc.sync.dma_start(out=bar_sb[:], in_=b_ar[:])
            bfc_sb = cpool.tile([P, NMF], F32, tag="bfc")
            nc.sync.dma_start(out=bfc_sb[:], in_=b_fc[:])

            # ------------- Phase 1: z = x @ W_ih^T + b_z (transposed) -------
            with (
                nc.named_scope("ph1_z"),
                tc.tile_pool(name="p1w", bufs=1) as p1w,
                tc.tile_pool(name="p1x", bufs=16) as p1x,
                tc.tile_pool(name="p1z", bufs=4) as p1z,
                tc.tile_pool(name="p1ps", bufs=6, space="PSUM") as p1ps,
            ):
                wih_sb = p1w.tile([P, NKF * HID], FP16, tag="wih")
                nc.sync.dma_start(
                    out=wih_sb.rearrange("p (k m) -> p k m", k=NKF),
                    in_=w_ih.rearrange("(k p) m -> p k m", p=P),
                )
                x_view = x_T.rearrange("(k p) t b -> p k t b", p=P)
                for tch in range(N_TCH_W):
                    t0 = tch * TC8
                    x_tiles = []
                    for k in range(NKF):
                        xt = p1x.tile([P, TC8 * B], FP16, tag="x")
                        nc.sync.dma_start(
                            out=xt.rearrange("p (t b) -> p t b", t=TC8),
                            in_=x_view[:, k, t0 : t0 + TC8, :],
                        )
                        x_tiles.append(xt)
                    for m in range(NMH):
                        acc = p1ps.tile([P, TC8 * B], F32, tag="zacc")
                        for k in range(NKF):
                            nc.tensor.matmul(
                                acc[:],
                                wih_sb[:, k * HID + m * P : k * HID + (m + 1) * P],
                                x_tiles[k][:],
                                start=(k == 0),
                                stop=(k == NKF - 1),
                            )
                        zt = p1z.tile([P, TC8 * B], FP16, tag="zt")
                        nc.vector.tensor_scalar_add(zt[:], acc[:], bz_sb[:, m : m + 1])
                        nc.sync.dma_start(
                            out=z_dram[m][:, t0 : t0 + TC8, :],
                            in_=zt.rearrange("p (t b) -> p t b", t=TC8),
                        )

            # ------------- Phase 2+3: recurrence ---------------------------
            with (
                tc.tile_pool(name="p2w", bufs=2) as p2w,
                tc.tile_pool(name="p2h", bufs=3) as p2h,
                tc.tile_pool(name="p2z", bufs=4) as p2z,
                tc.tile_pool(name="p2ps", bufs=8, space="PSUM") as p2ps,
            ):
                whh_sb = p2w.tile([P, NKH * HID], FP16, tag="w")
                nc.sync.dma_start(
                    out=whh_sb.rearrange("p (k m) -> p k m", k=NKH),
                    in_=w_hh.rearrange("(k p) m -> p k m", p=P),
                )
                war_sb = p2w.tile([P, NKH * HID], FP16, tag="w")
                nc.sync.dma_start(
                    out=war_sb.rearrange("p (k m) -> p k m", k=NKH),
                    in_=w_ar.rearrange("(k p) m -> p k m", p=P),
                )

                h_prev = p2h.tile([P, NMH * B], FP16, tag="h")
                nc.vector.memset(h_prev[:], 0.0)

                def step(h_prev, wt, z_t=None, store_t=None):
                    h_cur = p2h.tile([P, NMH * B], FP16, tag="h")
                    for m in range(NMH):
                        acc = p2ps.tile([P, B], F32, tag="acc")
                        for k in range(NKH):
                            nc.tensor.matmul(
                                acc[:],
                                wt[:, k * HID + m * P : k * HID + (m + 1) * P],
                                h_prev[:, k * B : (k + 1) * B],
                                start=(k == 0),
                                stop=(k == NKH - 1),
                            )
                        out_sl = h_cur[:, m * B : (m + 1) * B]
                        if z_t is not None:
                            nc.vector.tensor_tensor(
                                out_sl, acc[:], z_t[:, m * B : (m + 1) * B],
                                op=mybir.AluOpType.add,
                            )
                            if m == NMH - 1:
                                # one batched in-place relu over the full h
                                nc.vector.tensor_scalar_max(h_cur[:], h_cur[:], 0.0)
                        else:
                            nc.vector.tensor_scalar(
                                out_sl, acc[:], bar_sb[:, m : m + 1], 0.0,
                                op0=mybir.AluOpType.add, op1=mybir.AluOpType.max,
                            )
                    if store_t is not None:
                        nc.sync.dma_start(
                            out=h_step_view[:, store_t, :, :],
                            in_=h_cur.rearrange("p (m b) -> p m b", m=NMH),
                        )
                    return h_cur

                with nc.named_scope("ph2_warm"):
                    for t in range(T_WARM):
                        z_t = p2z.tile([P, NMH * B], FP16, tag="z")
                        nc.sync.dma_start(
                            out=z_t.rearrange("p (m b) -> p m b", m=NMH),
                            in_=z_step_view[:, t, :, :],
                        )
                        h_prev = step(h_prev, whh_sb, z_t=z_t)

                nc.sync.dma_start(
                    out=h_step_view[:, 0, :, :],
                    in_=h_prev.rearrange("p (m b) -> p m b", m=NMH),
                )

                with nc.named_scope("ph3_ar"):
                    for t in range(T_AR):
                        h_prev = step(h_prev, war_sb, z_t=None, store_t=t + 1)

            # ------------- Phase 4: preds = h_hist @ W_fc^T + b_fc ----------
            with (
                nc.named_scope("ph4_fc"),
                tc.tile_pool(name="p4w", bufs=1) as p4w,
                tc.tile_pool(name="p4h", bufs=24) as p4h,
                tc.tile_pool(name="p4o", bufs=4) as p4o,
                tc.tile_pool(name="p4ps", bufs=6, space="PSUM") as p4ps,
            ):
                wfc_sb = p4w.tile([P, NKH * FEAT], FP16, tag="wfc")
                nc.sync.dma_start(
                    out=wfc_sb.rearrange("p (k m) -> p k m", k=NKH),
                    in_=w_fc.rearrange("(k p) m -> p k m", p=P),
                )
                out_view = out.rearrange("(mp p) t b -> p mp t b", p=P)
                # 129 = 8 chunks of 16 + 1
                chunks = [(i * TC8, TC8) for i in range(8)] + [(8 * TC8, 1)]
                for t0, tn in chunks:
                    h_tiles = []
                    for k in range(NKH):
                        ht = p4h.tile([P, TC8 * B], FP16, tag="hh")
                        nc.sync.dma_start(
                            out=ht.rearrange("p (t b) -> p t b", t=TC8)[:, :tn, :],
                            in_=h_dram[k][:, t0 : t0 + tn, :],
                        )
                        h_tiles.append(ht)
                    for mp in range(NMF):
                        acc = p4ps.tile([P, TC8 * B], F32, tag="oacc")
                        for k in range(NKH):
                            nc.tensor.matmul(
                                acc[:, : tn * B],
                                wfc_sb[:, k * FEAT + mp * P : k * FEAT + (mp + 1) * P],
                                h_tiles[k][:, : tn * B],
                                start=(k == 0),
                                stop=(k == NKH - 1),
                            )
                        ot = p4o.tile([P, TC8 * B], F32, tag="ot")
                        nc.vector.tensor_scalar_add(
                            ot[:, : tn * B], acc[:, : tn * B], bfc_sb[:, mp : mp + 1]
                        )
                        nc.sync.dma_start(
                            out=out_view[:, mp, t0 : t0 + tn, :],
                            in_=ot.rearrange("p (t b) -> p t b", t=TC8)[:, :tn, :],
                        )
    return nc


_cached_nc = None


def _get_nc():
    global _cached_nc
    if _cached_nc is None:
        _cached_nc = _build_nc()
    return _cached_nc


def _prep_inputs(inputs, W_ih, b_ih, W_hh, b_hh, W_fc, b_fc):
    inputs = np.asarray(inputs, dtype=np.float32)
    W_ih = np.asarray(W_ih, dtype=np.float32)
    W_hh = np.asarray(W_hh, dtype=np.float32)
    W_fc = np.asarray(W_fc, dtype=np.float32)
    b_ih = np.asarray(b_ih, dtype=np.float32)
    b_hh = np.asarray(b_hh, dtype=np.float32)
    b_fc = np.asarray(b_fc, dtype=np.float32)

    W_ar = W_hh + W_ih @ W_fc
    b_arr = b_ih + b_hh + W_ih @ b_fc
    b_zz = b_ih + b_hh

    common = {
        "w_ih": np.ascontiguousarray(W_ih.T).astype(np.float16),
        "w_hh": np.ascontiguousarray(W_hh.T).astype(np.float16),
        "w_ar": np.ascontiguousarray(W_ar.T).astype(np.float16),
        "w_fc": np.ascontiguousarray(W_fc.T).astype(np.float16),
        "b_z": np.ascontiguousarray(b_zz.reshape(NMH, P).T),
        "b_ar": np.ascontiguousarray(b_arr.reshape(NMH, P).T),
        "b_fc": np.ascontiguousarray(b_fc.reshape(NMF, P).T),
    }
    in_maps = []
    for c in range(N_CORES):
        xc = inputs[c * B : (c + 1) * B]                      # [32, 128, 1024]
        x_T = np.ascontiguousarray(xc.transpose(2, 1, 0)).astype(np.float16)
        in_maps.append({"x_T": x_T, **common})
    return in_maps


def _assemble(results):
    outs = []
    for c in range(N_CORES):
        o = results[c]["out"]                                # [1024, 129, 32] f32
        outs.append(np.ascontiguousarray(o.transpose(2, 1, 0)))  # [32, 129, 1024]
    return np.concatenate(outs, axis=0)


def run(trace=False, **inputs):
    in_maps = _prep_inputs(**inputs)
    nc = _get_nc()
    res = run_bass_kernel_spmd(nc, in_maps, list(range(N_CORES)), trace=trace)
    return _assemble(res.results), res


def kernel(**inputs):
    out, _ = run(trace=False, **inputs)
    return out
